# revision 8
# baseline (speedup 1.0000x reference)
"""Non-Local Block (NLB) kernel for 8 Trainium2 NeuronCores.

Reference computation (B=2, C=256, Ci=128, H=W=80, N=H*W=6400):
    theta = theta_w @ x + theta_b    [B, Ci, N]   (1x1 conv)
    phi   = phi_w @ x + phi_b        [B, Ci, N]
    g     = g_w @ x + g_b            [B, Ci, N]
    f[b, n, m] = theta[b,:,n] . phi[b,:,m]
    f_div = softmax(f, axis=1)       (over n -- the *query/row* axis!)
    y[b, n, :] = sum_m f_div[b,n,m] * g[b,:,m]
    out = W_w @ y + W_b + x

Sharding: 8 cores = 2 batches x 4 shards of the *column* (m) axis of f.
The softmax reduction axis (n) stays fully local per column, so there are
no collectives: each core computes a partial y/W_y over its local m range
and the host sums the 4 partials per batch.

Per-core layout trick: f is computed transposed, f_T[m, n], so the
softmax reduction over n is a free-axis reduction.  The exp is fused with
the column sum via ScalarE activation(Exp, accum_out=...) reading the
matmul PSUM directly.  The division by the softmax denominator S[m] is
folded into g (y = sum_m exp_f[m,n] * (g[m,c]/S[m])), making y one plain
matmul chain.  No max-subtraction is needed: |f| < ~45, well within fp32
exp range, and softmax is shift-invariant so this matches the reference.

dtypes: f-path matmuls run in float32r (full-speed fp32 PE mode),
exp_f/g in bf16, y/W accumulate in fp32 PSUM.
"""

import numpy as np

B = 2
CH = 256
CI = 128
N = 6400
NL = N // 4  # 1600 columns of f per core
P = 128

# m tiles within a core's 1600-column shard: 12 full 128-tiles + one 64-tile
M_TILES = [(i * P, P) for i in range(12)] + [(12 * P, 64)]
# pipeline instances (groups of m-tiles) so exp (ScalarE) of later groups
# overlaps y-matmuls (TensorE) of earlier groups
INSTANCES = [(0, 5), (5, 9), (9, 13)]
# f / exp chunking: PSUM double-buffer of 3 banks (1536 fp32) each
F_CHUNKS = [(0, 1536), (1536, 1536), (3072, 1536), (4608, 1536), (6144, 256)]
# 512-wide chunks for theta / y / W (one PSUM bank)
N_CHUNKS = [(j * 512, 512) for j in range(12)] + [(6144, 256)]
PHI_CHUNKS = [(0, 512), (512, 512), (1024, 512), (1536, 64)]

_COMPILED = None


def _build():
    import concourse.bass as bass
    import concourse.tile as tile
    from concourse import bacc, mybir

    f32 = mybir.dt.float32
    f32r = mybir.dt.float32r
    bf16 = mybir.dt.bfloat16
    EXP = mybir.ActivationFunctionType.Exp

    nc = bacc.Bacc("TRN2", target_bir_lowering=False, debug=False, num_devices=8)

    def din(name, shape, dt=f32):
        return nc.dram_tensor(name, shape, dt, kind="ExternalInput").ap()

    x_d = din("x", [CH, N], f32r)
    xs_d = din("xs", [CH, NL], f32r)
    xsh_d = din("xsh", [CH, NL], bf16)
    thetaT_d = din("thetaT", [CH, CI], f32r)
    phiT_d = din("phiT", [CH, CI], f32r)
    phib_d = din("phib", [CI, 1])
    gT_d = din("gT", [CH, CI], bf16)
    gbrow_d = din("gbrow", [1, CI], bf16)
    WT_d = din("WT", [CI, CH], f32r)
    wy_d = nc.dram_tensor("wy", [CH, N], f32, kind="ExternalOutput").ap()

    def r(ap):  # operands are already float32r-typed
        return ap

    from contextlib import ExitStack

    with tile.TileContext(nc) as tc, ExitStack() as ctx:
        cst = ctx.enter_context(tc.tile_pool(name="cst", bufs=1))
        seq = ctx.enter_context(tc.tile_pool(name="seq", bufs=1))
        xpool = ctx.enter_context(tc.tile_pool(name="xp", bufs=6))
        expf_pool = ctx.enter_context(tc.tile_pool(name="ef", bufs=9))
        stats = ctx.enter_context(tc.tile_pool(name="st", bufs=13))
        wyp = ctx.enter_context(tc.tile_pool(name="wyp", bufs=2))
        pf = ctx.enter_context(tc.tile_pool(name="pf", bufs=2, space="PSUM"))
        ps = ctx.enter_context(tc.tile_pool(name="ps", bufs=2, space="PSUM"))

        # ---- constants ----
        thetaT_t = [cst.tile([P, CI], f32r, tag=f"wth{h}", name=f"thetaT_{h}") for h in range(2)]
        phiT_t = [cst.tile([P, CI], f32r, tag=f"wph{h}", name=f"phiT_{h}") for h in range(2)]
        gT_t = [cst.tile([P, CI], bf16, tag=f"wg{h}", name=f"gT_{h}") for h in range(2)]
        for h in range(2):
            nc.sync.dma_start(thetaT_t[h][:], thetaT_d[h * P : (h + 1) * P, :])
            nc.sync.dma_start(phiT_t[h][:], phiT_d[h * P : (h + 1) * P, :])
            nc.sync.dma_start(gT_t[h][:], gT_d[h * P : (h + 1) * P, :])
        WT_t = cst.tile([CI, CH], f32r, tag="wt")
        nc.sync.dma_start(WT_t[:], WT_d[:])
        phib_t = cst.tile([CI, 1], f32, tag="pb")
        nc.sync.dma_start(phib_t[:], phib_d[:])
        gbrow_t = cst.tile([1, CI], bf16, tag="gb")
        nc.sync.dma_start(gbrow_t[:], gbrow_d[:])
        ones1_t = cst.tile([1, P], bf16, tag="on")
        nc.vector.memset(ones1_t[:], 1.0)

        # ---- persistent sequence tiles ----
        theta_x = seq.tile([P, N], f32r, tag="th")
        phi_x = seq.tile([P, NL], f32r, tag="ph")
        g_x = seq.tile([P, 13 * P], f32, tag="gx")
        g_sc = seq.tile([P, 13 * P], bf16, tag="gs")
        yacc = seq.tile([P, N], f32r, tag="ya")

        # ---- theta projection (streamed over x chunks); no bias needed:
        # a constant shift of theta along the softmax axis cancels exactly ----
        for noff, nsz in N_CHUNKS:
            xt = [xpool.tile([P, 512], f32r, tag="x", name=f"xt_{h}") for h in range(2)]
            for h in range(2):
                nc.sync.dma_start(
                    xt[h][:, :nsz], x_d[h * P : (h + 1) * P, noff : noff + nsz]
                )
            pt = ps.tile([P, 512], f32, tag="p5")
            nc.tensor.matmul(
                pt[:, :nsz], r(thetaT_t[0][:]), r(xt[0][:, :nsz]), start=True, stop=False
            )
            nc.tensor.matmul(
                pt[:, :nsz], r(thetaT_t[1][:]), r(xt[1][:, :nsz]), start=False, stop=True
            )
            nc.vector.tensor_copy(theta_x[:, noff : noff + nsz], pt[:, :nsz])

        # ---- phi projection on the local column slice (+bias) ----
        for noff, nsz in PHI_CHUNKS:
            xst = [xpool.tile([P, 512], f32r, tag="x", name=f"xst_{h}") for h in range(2)]
            for h in range(2):
                nc.sync.dma_start(
                    xst[h][:, :nsz], xs_d[h * P : (h + 1) * P, noff : noff + nsz]
                )
            pt = ps.tile([P, 512], f32, tag="p5")
            nc.tensor.matmul(
                pt[:, :nsz],
                r(phiT_t[0][:]),
                r(xst[0][:, :nsz]),
                start=True,
                stop=False,
            )
            nc.tensor.matmul(
                pt[:, :nsz],
                r(phiT_t[1][:]),
                r(xst[1][:, :nsz]),
                start=False,
                stop=True,
            )
            nc.vector.tensor_scalar_add(
                phi_x[:, noff : noff + nsz], pt[:, :nsz], phib_t[:]
            )

        # ---- g projection in [m, c] layout (bf16), bias via K=1 ones-row ----
        xsh_t = [seq.tile([P, NL], bf16, tag=f"xsh{h}", name=f"xsh_{h}") for h in range(2)]
        for h in range(2):
            nc.sync.dma_start(xsh_t[h][:], xsh_d[h * P : (h + 1) * P, :])
        for i, (moff, mt) in enumerate(M_TILES):
            pg = ps.tile([P, 512], f32, tag="p5")
            nc.tensor.matmul(
                pg[:mt, :CI],
                xsh_t[0][:, moff : moff + mt],
                gT_t[0][:],
                start=True,
                stop=False,
            )
            nc.tensor.matmul(
                pg[:mt, :CI],
                xsh_t[1][:, moff : moff + mt],
                gT_t[1][:],
                start=False,
                stop=False,
            )
            nc.tensor.matmul(
                pg[:mt, :CI], ones1_t[:1, :mt], gbrow_t[:1, :], start=False, stop=True
            )
            nc.vector.tensor_copy(g_x[:mt, i * P : i * P + CI], pg[:mt, :CI])

        # ---- f = phi^T theta (transposed layout), exp fused with column sum ----
        expf_tiles = []
        for i, (moff, mt) in enumerate(M_TILES):
            ef = expf_pool.tile([P, N], bf16, tag="ef")
            sparts = stats.tile([P, 8], f32, tag="sp")
            for jc, (noff, nsz) in enumerate(F_CHUNKS):
                pft = pf.tile([P, 1536], f32, tag="pf")
                for s in range(0, nsz, 512):
                    ssz = min(512, nsz - s)
                    nc.tensor.matmul(
                        pft[:mt, s : s + ssz],
                        r(phi_x[:, moff : moff + mt]),
                        r(theta_x[:, noff + s : noff + s + ssz]),
                        start=True,
                        stop=True,
                    )
                nc.scalar.activation(
                    ef[:mt, noff : noff + nsz],
                    pft[:mt, :nsz],
                    EXP,
                    accum_out=sparts[:mt, jc : jc + 1],
                )
            s_i = stats.tile([P, 1], f32, tag="si")
            r_i = stats.tile([P, 1], f32, tag="ri")
            nc.vector.tensor_reduce(
                s_i[:mt, :], sparts[:mt, 0:5], mybir.AxisListType.X, mybir.AluOpType.add
            )
            nc.vector.reciprocal(r_i[:mt, :], s_i[:mt, :])
            nc.vector.tensor_scalar_mul(
                g_sc[:mt, i * P : i * P + CI], g_x[:mt, i * P : i * P + CI], r_i[:mt, :]
            )
            expf_tiles.append(ef)

        # ---- y^T[c, n] = sum_m g_sc[m, c] * exp_f[m, n], instance-pipelined ----
        for inst_idx, (i0, i1) in enumerate(INSTANCES):
            for noff, nsz in N_CHUNKS:
                py = ps.tile([P, 512], f32, tag="p5")
                for i in range(i0, i1):
                    moff, mt = M_TILES[i]
                    nc.tensor.matmul(
                        py[:, :nsz],
                        g_sc[:mt, i * P : i * P + CI],
                        expf_tiles[i][:mt, noff : noff + nsz],
                        start=(i == i0),
                        stop=(i == i1 - 1),
                    )
                if inst_idx == 0:
                    nc.vector.tensor_copy(yacc[:, noff : noff + nsz], py[:, :nsz])
                else:
                    nc.vector.tensor_tensor(
                        yacc[:, noff : noff + nsz],
                        py[:, :nsz],
                        yacc[:, noff : noff + nsz],
                        mybir.AluOpType.add,
                    )

        # ---- W_y^T[co, n] = W^T yacc ----
        for h in range(2):
            for noff, nsz in N_CHUNKS:
                pw = ps.tile([P, 512], f32, tag="p5")
                nc.tensor.matmul(
                    pw[:, :nsz],
                    r(WT_t[:, h * P : (h + 1) * P]),
                    r(yacc[:, noff : noff + nsz]),
                    start=True,
                    stop=True,
                )
                wt = wyp.tile([P, 512], f32, tag="wy")
                nc.vector.tensor_copy(wt[:, :nsz], pw[:, :nsz])
                nc.sync.dma_start(
                    wy_d[h * P : (h + 1) * P, noff : noff + nsz], wt[:, :nsz]
                )

    nc.compile()
    return nc


def _get_compiled():
    global _COMPILED
    if _COMPILED is None:
        _COMPILED = _build()
    return _COMPILED


def _round_f32r(a):
    """Round fp32 to the nearest value representable as hi+lo bf16 pair."""
    import ml_dtypes

    a = np.asarray(a, np.float32)
    hi = a.astype(ml_dtypes.bfloat16).astype(np.float32)
    lo = (a - hi).astype(ml_dtypes.bfloat16).astype(np.float32)
    return hi + lo


def kernel(x, g_w, g_b, theta_w, theta_b, phi_w, phi_b, W_w, W_b):
    import ml_dtypes
    from concourse.bass_utils import run_bass_kernel_spmd

    nc = _get_compiled()

    x = _round_f32r(np.ascontiguousarray(np.asarray(x, np.float32)).reshape(B, CH, N))
    thetaT = _round_f32r(np.ascontiguousarray(np.asarray(theta_w, np.float32).T))
    phiT = _round_f32r(np.ascontiguousarray(np.asarray(phi_w, np.float32).T))
    gT = np.ascontiguousarray(np.asarray(g_w, np.float32).T).astype(ml_dtypes.bfloat16)
    WT = _round_f32r(np.ascontiguousarray(np.asarray(W_w, np.float32).T))
    phib = np.asarray(phi_b, np.float32).reshape(CI, 1)
    gbrow = np.asarray(g_b, np.float32).reshape(1, CI).astype(ml_dtypes.bfloat16)

    in_maps = []
    for core in range(8):
        b, q = divmod(core, 4)
        xs = np.ascontiguousarray(x[b][:, q * NL : (q + 1) * NL])
        in_maps.append(
            {
                "x": x[b],
                "xs": xs,
                "xsh": xs.astype(ml_dtypes.bfloat16),
                "thetaT": thetaT,
                "phiT": phiT,
                "phib": phib,
                "gT": gT,
                "gbrow": gbrow,
                "WT": WT,
            }
        )

    res = run_bass_kernel_spmd(nc, in_maps, list(range(8)))

    out = np.empty((B, CH, N), np.float32)
    for b in range(B):
        acc = res.results[4 * b]["wy"].astype(np.float32)
        for q in range(1, 4):
            acc = acc + res.results[4 * b + q]["wy"]
        out[b] = acc + np.asarray(W_b, np.float32)[:, None] + x[b]
    return out.reshape(B, CH, 80, 80)
